# revision 2
# baseline (speedup 1.0000x reference)
"""Trainium2 Bass kernel for nn_Decoder_78305843741218 — fp8 DoubleRow version.

2-layer GRU decoder, autoregressive over T=256 steps, batch 1024.
Sharding: data-parallel over batch -> 128 samples/core on 8 cores.

Design:
  - All GEMMs run fp8-e4m3 with perf_mode=DoubleRow: stationary is the
    transposed activations as [Ki=128, 2, M] k-pair tiles, moving is the
    weights as [Ki=128, 2, N] k-pair tiles -> K=256 contraction per MM at
    2x PE throughput.  Power-of-2 scales keep e4m3 in range: W*128,
    h*16, x*32 (every PSUM group is scaled by ST=2048, divided back out
    via activation `scale`).  Numpy-simulated rel err 5.7e-3 (tol 2e-2).
  - All weights SBUF-resident at fp8 (10.4 MB) -> no W_hh1 streaming.
  - Software-pipelined body: iteration t emits
      L1(t) -> [cell1] -> L0h(t+1) -> FC(t) -> [fc act, x^T] -> L0x(t+1)
      -> [cell0(t+1)]
    so the PE covers the cell1/fc chains with next-step L0 work.
  - Host-precomputed glob@W_ih0 consts (c0rz/c0n, x ST, bf16) enter PSUM
    on the PE via identity-stationary matmuls that open each L0 group;
    layer-1/hn biases enter via bf16 rank-1 (ones x row) matmuls.
  - Cell math: h' = sigmoid(-pre_z)*n + z*h with z*h on GpSimd
    (off-chain), tanh direct (sigmoid_and_others table has both), state
    stored as SH*h in bf16, chunked 4x256 so chains run at quarter width
    and each k-pair's DMA transposes + fp8 convert start immediately.
  - All recurrent tiles are split per k-pair (separate memrefs) for
    fine-grained Tile deps; h0 transposes ride the SP hwdge queue, h1
    the ACT queue, x the SP queue (transpose-only queues, no xbar-mode
    flips).

Output is stored T-major (T*128, D) per core; host reassembles.
"""

import os
import numpy as np
import ml_dtypes

H = 1024
D = 256
T = 256
P = 128
NCORES = 8
KT2 = 4              # DoubleRow k-pair tiles for the H contraction
CW = H // KT2        # 256-col state chunk = one k-pair of columns
CH_N = 2             # cell chunks: 2x512 won the TimelineSim sweep
CCW = H // CH_N      # cell chunk width
U_POOL = True        # z*h on GpSimd (off the critical chain)
FC_EARLY = False
SW = 128.0           # weight scale (fp8)
SH = 16.0            # hidden-state scale (fp8)
SX = 32.0            # x scale (fp8)
ST = SW * SH         # psum scale of every accumulation group (= SX * 64)

_prog_cache = {}


def _build_program(t_steps=T, unroll=8):
    import concourse.bass as bass
    import concourse.bacc as bacc
    import concourse.mybir as mybir
    import concourse.tile as tile
    from contextlib import ExitStack

    f32 = mybir.dt.float32
    bf16 = mybir.dt.bfloat16
    f8 = mybir.dt.float8e4
    AF = mybir.ActivationFunctionType
    ALU = mybir.AluOpType
    DR = mybir.MatmulPerfMode.DoubleRow

    full_unroll = unroll >= t_steps
    if not full_unroll:
        assert t_steps % unroll == 0
    n_iter = 1 if full_unroll else t_steps // unroll

    nc = bacc.Bacc(None, target_bir_lowering=False)

    # ---- I/O ----
    d_whh0 = nc.dram_tensor("whh0", (P, KT2, 2, 3 * H), f8, kind="ExternalInput")
    d_wih1 = nc.dram_tensor("wih1", (P, KT2, 2, 3 * H), f8, kind="ExternalInput")
    d_whh1 = nc.dram_tensor("whh1", (P, KT2, 2, 3 * H), f8, kind="ExternalInput")
    d_wih0x = nc.dram_tensor("wih0x", (P, 2, 3 * H), f8, kind="ExternalInput")
    d_wfc = nc.dram_tensor("wfc", (P, KT2, 2, D), f8, kind="ExternalInput")
    d_h0s = nc.dram_tensor("h0s_init", (P, H), bf16, kind="ExternalInput")
    d_h1s = nc.dram_tensor("h1s_init", (P, H), bf16, kind="ExternalInput")
    d_h0t8 = nc.dram_tensor("h0t8_init", (P, KT2, 2, P), f8, kind="ExternalInput")
    d_h1t8 = nc.dram_tensor("h1t8_init", (P, KT2, 2, P), f8, kind="ExternalInput")
    d_x0t8 = nc.dram_tensor("x0t8_init", (P, 2, P), f8, kind="ExternalInput")
    d_c0rz = nc.dram_tensor("c0rz", (P, 2 * H), bf16, kind="ExternalInput")
    d_c0n = nc.dram_tensor("c0n", (P, H), bf16, kind="ExternalInput")
    d_bhh0n = nc.dram_tensor("bhh0n", (1, H), bf16, kind="ExternalInput")
    d_brz1 = nc.dram_tensor("brz1", (1, 2 * H), bf16, kind="ExternalInput")
    d_bih1n = nc.dram_tensor("bih1n", (1, H), bf16, kind="ExternalInput")
    d_bhh1n = nc.dram_tensor("bhh1n", (1, H), bf16, kind="ExternalInput")
    d_bfc = nc.dram_tensor("bfc", (1, D), bf16, kind="ExternalInput")
    d_ident = nc.dram_tensor("ident", (P, P), bf16, kind="ExternalInput")
    d_res = nc.dram_tensor("res", (t_steps * P, D), f32, kind="ExternalOutput")

    with tile.TileContext(nc) as tc, ExitStack() as ctx:
        const = ctx.enter_context(tc.tile_pool(name="const", bufs=1))
        act = ctx.enter_context(tc.tile_pool(name="act", bufs=2))
        ps = ctx.enter_context(tc.tile_pool(name="ps", bufs=1, space="PSUM"))

        _gc = [0]

        def gload(shape, dtype, src, name=None):
            _gc[0] += 1
            t = const.tile(shape, dtype, name=name or f"cst{_gc[0]}")
            nc.scalar.dma_start(t, src)
            return t

        whh0 = gload([P, KT2, 2, 3 * H], f8, d_whh0[:])
        wih1 = gload([P, KT2, 2, 3 * H], f8, d_wih1[:])
        whh1 = gload([P, KT2, 2, 3 * H], f8, d_whh1[:])
        wih0x = gload([P, 2, 3 * H], f8, d_wih0x[:])
        wfc = gload([P, KT2, 2, D], f8, d_wfc[:])
        c0rz = gload([P, 2 * H], bf16, d_c0rz[:])
        c0n = gload([P, H], bf16, d_c0n[:])
        bhh0n = gload([1, H], bf16, d_bhh0n[:])
        brz1 = gload([1, 2 * H], bf16, d_brz1[:])
        bih1n = gload([1, H], bf16, d_bih1n[:])
        bhh1n = gload([1, H], bf16, d_bhh1n[:])
        bfc = gload([1, D], bf16, d_bfc[:])
        ident = gload([P, P], bf16, d_ident[:])
        ones = const.tile([1, P], bf16)
        nc.vector.memset(ones, 1.0)

        # per-k-pair recurrent state (separate memrefs: fine-grained deps)
        h0s = [gload([P, CW], bf16, d_h0s[:, j * CW:(j + 1) * CW], name=f"h0s{j}")
               for j in range(KT2)]
        h1s = [gload([P, CW], bf16, d_h1s[:, j * CW:(j + 1) * CW], name=f"h1s{j}")
               for j in range(KT2)]
        h0t8 = [gload([P, 2, P], f8, d_h0t8[:, j], name=f"h0t8_{j}")
                for j in range(KT2)]
        h1t8 = [gload([P, 2, P], f8, d_h1t8[:, j], name=f"h1t8_{j}")
                for j in range(KT2)]
        xt8 = gload([P, 2, P], f8, d_x0t8[:], name="xt8")
        h0tb = [const.tile([P, 2, P], bf16, name=f"h0tb{j}") for j in range(KT2)]
        h1tb = [const.tile([P, 2, P], bf16, name=f"h1tb{j}") for j in range(KT2)]
        xtb = const.tile([P, 2, P], bf16, name="xtb")

        def rank1(ps_t, row, n, start):
            for c in range((n + 511) // 512):
                w = min(512, n - c * 512)
                nc.tensor.matmul(ps_t[:, c * 512:c * 512 + w], ones,
                                 row[:, c * 512:c * 512 + w],
                                 start=start, stop=False)

        def identmm(ps_t, full, g, start):
            """Add the per-sample bf16 const `full[:, g:g+H]` into psum via
            identity-stationary matmuls (frees DVE + shortens cell chains)."""
            for c in range(2):
                nc.tensor.matmul(ps_t[:, c * 512:(c + 1) * 512], ident,
                                 full[:, g + c * 512:g + (c + 1) * 512],
                                 start=start, stop=False)

        def mm2(ps_t, lhsT, w, g, start, stop):
            """Two 512-col DoubleRow matmuls covering one gate's H columns."""
            for c in range(2):
                nc.tensor.matmul(
                    ps_t[:, c * 512:(c + 1) * 512], lhsT,
                    w[:, :, g + c * 512:g + (c + 1) * 512],
                    start=start, stop=stop,
                    perf_mode=DR)

        def transpose_pair(q, src_bf, dst_bf, dst_f8):
            """Transpose one 256-col scaled-bf16 state chunk into dst_bf and
            convert to the fp8 k-pair stationary tile on GpSimd."""
            for o in range(2):
                q.dma_start_transpose(dst_bf[:, o], src_bf[:, o * P:(o + 1) * P])
            nc.gpsimd.tensor_copy(dst_f8[:, :], dst_bf[:, :])

        def emit_l0(ps_r, ps_z, ps_hn):
            # consts/biases open the groups; then r/z/hn from W_hh0 x h0^T
            identmm(ps_r, c0rz, 0, start=True)
            identmm(ps_z, c0rz, H, start=True)
            rank1(ps_hn, bhh0n, H, start=True)
            for j in range(KT2):
                lhsT = h0t8[j][:, :]
                w_j = whh0[:, j]
                mm2(ps_r, lhsT, w_j, 0, False, False)
                mm2(ps_z, lhsT, w_j, H, False, False)
                mm2(ps_hn, lhsT, w_j, 2 * H, False, j == KT2 - 1)

        def emit_l0x(ps_r, ps_z, ps_in):
            # x-part of L0: closes r and z; opens (with c0n) + closes in
            identmm(ps_in, c0n, 0, start=True)
            lhsT = xt8[:, :]
            mm2(ps_r, lhsT, wih0x, 0, False, True)
            mm2(ps_z, lhsT, wih0x, H, False, True)
            mm2(ps_in, lhsT, wih0x, 2 * H, False, True)

        def emit_cell(ps_r, ps_z, ps_hn, ps_in, hs, htb, ht8, q):
            """PSUM r/z/hn/in (x ST) -> hs (bf16, x SH, in place) -> ht8.

            h' = sigmoid(-pre_z)*n + z*h, chunked 4x256: chains run at
            quarter width, each k-pair transposes as soon as it's ready.
            z*h rides GpSimd (off the critical chain).
            """
            rz = act.tile([P, 2 * H], f32, tag="rz", bufs=2)
            zc = act.tile([P, H], f32, tag="zc", bufs=2)
            g1 = act.tile([P, H], f32, tag="g1", bufs=2)
            u = act.tile([P, H], bf16, tag="u", bufs=2)
            nt = act.tile([P, H], f32, tag="n_sb", bufs=2)
            vs = act.tile([P, H], f32, tag="vs", bufs=2)
            kpc = KT2 // CH_N  # k-pairs per cell chunk
            for c in range(CH_N):
                s = slice(c * CCW, (c + 1) * CCW)
                zs = slice(H + c * CCW, H + (c + 1) * CCW)
                nc.scalar.activation(rz[:, s], ps_r[:, s], AF.Sigmoid,
                                     scale=1.0 / ST)
                nc.scalar.activation(rz[:, zs], ps_z[:, s], AF.Sigmoid,
                                     scale=1.0 / ST)
                nc.scalar.activation(zc[:, s], ps_z[:, s], AF.Sigmoid,
                                     scale=-1.0 / ST)
                nc.vector.tensor_mul(g1[:, s], rz[:, s], ps_hn[:, s])
                for k in range(kpc):
                    j = c * kpc + k
                    js = slice(H + j * CW, H + (j + 1) * CW)
                    ue = nc.gpsimd if U_POOL else nc.vector
                    ue.tensor_mul(u[:, j * CW:(j + 1) * CW], rz[:, js], hs[j])
                nc.vector.tensor_add(nt[:, s], g1[:, s], ps_in[:, s])
                nc.scalar.activation(nt[:, s], nt[:, s], AF.Tanh,
                                     scale=1.0 / ST)
                nc.vector.scalar_tensor_tensor(vs[:, s], zc[:, s], SH, nt[:, s],
                                               ALU.mult, ALU.mult)
                for k in range(kpc):
                    j = c * kpc + k
                    jd = slice(j * CW, (j + 1) * CW)
                    nc.vector.tensor_add(hs[j], vs[:, jd], u[:, jd])
                    transpose_pair(q, hs[j], htb[j], ht8[j])

        def emit_l1():
            ps_r1 = ps.tile([P, H], f32, tag="ps_r")
            ps_z1 = ps.tile([P, H], f32, tag="ps_z")
            ps_hn1 = ps.tile([P, H], f32, tag="ps_hn")
            rank1(ps_r1, brz1[:, :H], H, start=True)
            rank1(ps_z1, brz1[:, H:], H, start=True)
            rank1(ps_hn1, bhh1n, H, start=True)
            for j in range(KT2):
                lhsT = h1t8[j][:, :]
                w_j = whh1[:, j]
                mm2(ps_r1, lhsT, w_j, 0, False, False)
                mm2(ps_z1, lhsT, w_j, H, False, False)
                mm2(ps_hn1, lhsT, w_j, 2 * H, False, j == KT2 - 1)
            ps_in1 = ps.tile([P, H], f32, tag="ps_in")
            rank1(ps_in1, bih1n, H, start=True)
            for j in range(KT2):
                lhsT = h0t8[j][:, :]
                w_j = wih1[:, j]
                mm2(ps_r1, lhsT, w_j, 0, False, j == KT2 - 1)
                mm2(ps_z1, lhsT, w_j, H, False, j == KT2 - 1)
                mm2(ps_in1, lhsT, w_j, 2 * H, False, j == KT2 - 1)
            return ps_r1, ps_z1, ps_hn1, ps_in1

        def emit_fc_out(res_row0):
            ps_fc = ps.tile([P, D], f32, tag="ps_in")
            rank1(ps_fc, bfc, D, start=True)
            for j in range(KT2):
                nc.tensor.matmul(ps_fc, h1t8[j][:, :], wfc[:, j],
                                 start=False, stop=j == KT2 - 1,
                                 perf_mode=DR)
            xf = act.tile([P, D], f32, tag="xf", bufs=2)
            xbf = act.tile([P, D], bf16, tag="xbf", bufs=2)
            sigp = act.tile([P, 47], f32, tag="sigp", bufs=2)
            sign = act.tile([P, 47], f32, tag="sign", bufs=2)
            s12 = act.tile([P, 2], f32, tag="s12", bufs=2)
            r12 = act.tile([P, 2], f32, tag="r12", bufs=2)
            # softmax inputs first (they head the serial chain); sigmoid
            # region after -- its upper half feeds the early o=1 transpose.
            # exp(x) = sigmoid(x)/sigmoid(-x); softmax is scale-invariant.
            nc.scalar.activation(sigp, ps_fc[:, 0:47], AF.Sigmoid,
                                 scale=1.0 / ST)
            nc.scalar.activation(sign, ps_fc[:, 0:47], AF.Sigmoid,
                                 scale=-1.0 / ST)
            nc.scalar.activation(xf[:, 47:D], ps_fc[:, 47:D], AF.Sigmoid,
                                 scale=1.0 / ST)
            # upper x^T half only needs the sigmoid region: start it now
            nc.vector.tensor_scalar_mul(xbf[:, P:D], xf[:, P:D], SX)
            nc.sync.dma_start_transpose(xtb[:, 1], xbf[:, P:D])
            nc.vector.reciprocal(sign, sign)
            nc.vector.scalar_tensor_tensor(
                xf[:, 0:32], sigp[:, 0:32], 1.0, sign[:, 0:32],
                ALU.mult, ALU.mult, accum_out=s12[:, 0:1])
            nc.vector.scalar_tensor_tensor(
                xf[:, 32:47], sigp[:, 32:47], 1.0, sign[:, 32:47],
                ALU.mult, ALU.mult, accum_out=s12[:, 1:2])
            nc.vector.reciprocal(r12, s12)
            nc.vector.tensor_scalar_mul(xf[:, 0:32], xf[:, 0:32], r12[:, 0:1])
            nc.vector.tensor_scalar_mul(xf[:, 32:47], xf[:, 32:47], r12[:, 1:2])
            nc.vector.tensor_scalar_mul(xbf[:, 0:P], xf[:, 0:P], SX)
            nc.sync.dma_start_transpose(xtb[:, 0], xbf[:, 0:P])
            nc.gpsimd.tensor_copy(xt8[:, :], xtb[:, :])
            nc.gpsimd.dma_start(d_res[bass.ds(res_row0, P), :], xf)

        # ---- prologue: L0(0) + cell0(0) ----
        p_r = ps.tile([P, H], f32, tag="ps_r")
        p_z = ps.tile([P, H], f32, tag="ps_z")
        p_hn = ps.tile([P, H], f32, tag="ps_hn")
        emit_l0(p_r, p_z, p_hn)
        p_in = ps.tile([P, H], f32, tag="ps_in")
        emit_l0x(p_r, p_z, p_in)
        emit_cell(p_r, p_z, p_hn, p_in, h0s, h0tb, h0t8, nc.sync)

        def body(res_row0):
            """Emits L1(t) .. L0(t+1)/cell0(t+1) for one step t."""
            l1ps = emit_l1()
            emit_cell(*l1ps, h1s, h1tb, h1t8, nc.scalar)
            if FC_EARLY:
                emit_fc_out(res_row0)
            ps_r = ps.tile([P, H], f32, tag="ps_r")
            ps_z = ps.tile([P, H], f32, tag="ps_z")
            ps_hn = ps.tile([P, H], f32, tag="ps_hn")
            emit_l0(ps_r, ps_z, ps_hn)
            if not FC_EARLY:
                emit_fc_out(res_row0)
            ps_in = ps.tile([P, H], f32, tag="ps_in")
            emit_l0x(ps_r, ps_z, ps_in)
            emit_cell(ps_r, ps_z, ps_hn, ps_in, h0s, h0tb, h0t8, nc.sync)

        if full_unroll:
            for t in range(t_steps):
                body(t * P)
        else:
            et = mybir.EngineType
            with tc.For_i(0, n_iter, 1,
                          hint_engines=(et.PE, et.DVE, et.Activation,
                                        et.SP, et.Pool)) as iv:
                row_base = iv * (unroll * P)
                for j in range(unroll):
                    body(row_base + j * P)

    _dedupe_ldweights(nc, mybir)
    nc.finalize()
    return nc


def _dedupe_ldweights(nc, mybir):
    """Drop redundant back-to-back Ldweights of the same stationary tile."""
    import orjson
    removed = 0
    for func in nc.m.functions:
        for blk in func.blocks:
            last_key = None
            kept = []
            blk_removed = 0
            for inst in blk.instructions:
                if getattr(inst, "engine", None) == mybir.EngineType.PE:
                    d = orjson.loads(mybir.instruction_to_pretty_json_string(inst))
                    op = d.get("opcode")
                    if op == "Ldweights":
                        si = d.get("sync_info") or {}
                        key = orjson.dumps(
                            (d.get("ins"), d.get("tile_position"),
                             d.get("tile_size"), d.get("perf_mode"),
                             d.get("is_transpose")))
                        if (key == last_key and not si.get("on_wait")
                                and not si.get("on_update")):
                            removed += 1
                            blk_removed += 1
                            continue
                        last_key = key
                kept.append(inst)
            if blk_removed:
                blk.instructions[:] = kept
    return removed


def _host_prep(inputs):
    """Build per-core input maps."""
    bf = ml_dtypes.bfloat16
    f8 = ml_dtypes.float8_e4m3
    embed = np.ascontiguousarray(np.asarray(inputs["embed"], dtype=np.float32))
    dynamics = np.asarray(inputs["dynamics"], dtype=np.float32)
    W_ih0 = np.asarray(inputs["W_ih0"], dtype=np.float32)
    W_hh0 = np.asarray(inputs["W_hh0"], dtype=np.float32)
    b_ih0 = np.asarray(inputs["b_ih0"], dtype=np.float32)
    b_hh0 = np.asarray(inputs["b_hh0"], dtype=np.float32)
    W_ih1 = np.asarray(inputs["W_ih1"], dtype=np.float32)
    W_hh1 = np.asarray(inputs["W_hh1"], dtype=np.float32)
    b_ih1 = np.asarray(inputs["b_ih1"], dtype=np.float32)
    b_hh1 = np.asarray(inputs["b_hh1"], dtype=np.float32)
    W_fc = np.asarray(inputs["W_fc"], dtype=np.float32)
    b_fc = np.asarray(inputs["b_fc"], dtype=np.float32)

    glob = embed[:, :H]
    h0i = embed[:, H:2 * H]
    h1i = embed[:, 2 * H:3 * H]
    x0 = dynamics[:, 0, :]

    c0 = (glob.astype(np.float64) @ W_ih0[:, :H].T.astype(np.float64)).astype(np.float32)
    c0 += b_ih0
    c0rz = np.ascontiguousarray((c0[:, :2 * H] + b_hh0[:2 * H]) * ST).astype(bf)
    c0n = np.ascontiguousarray(c0[:, 2 * H:] * ST).astype(bf)

    def w_dr(wT, scale):
        # [K, N] -> [P, K/256, 2, N] fp8 k-pair tiles
        K, N = wT.shape
        t = (wT * scale).reshape(K // 256, 2, P, N).transpose(2, 0, 1, 3)
        return np.ascontiguousarray(t.astype(f8))

    shared = {
        "whh0": w_dr(W_hh0.T, SW),
        "wih1": w_dr(W_ih1.T, SW),
        "whh1": w_dr(W_hh1.T, SW),
        "wih0x": w_dr(np.ascontiguousarray(W_ih0[:, H:].T), ST / SX)[:, 0],
        "wfc": w_dr(W_fc.T, SW),
        "bhh0n": (b_hh0[2 * H:] * ST).reshape(1, H).astype(bf),
        "brz1": ((b_ih1 + b_hh1)[:2 * H] * ST).reshape(1, 2 * H).astype(bf),
        "bih1n": (b_ih1[2 * H:] * ST).reshape(1, H).astype(bf),
        "bhh1n": (b_hh1[2 * H:] * ST).reshape(1, H).astype(bf),
        "bfc": (b_fc * ST).reshape(1, D).astype(bf),
        "ident": np.eye(P, dtype=np.float32).astype(bf),
    }

    def a_dr(a, scale):
        # [P, F] activations -> [P, F/256, 2, P] fp8 transposed k-pair tiles
        F = a.shape[1]
        t = (a * scale).T.reshape(F // 256, 2, P, P).transpose(2, 0, 1, 3)
        return np.ascontiguousarray(t.astype(f8))

    in_maps = []
    for c in range(NCORES):
        s = slice(c * P, (c + 1) * P)
        m = dict(shared)
        m["h0s_init"] = np.ascontiguousarray(h0i[s] * SH).astype(bf)
        m["h1s_init"] = np.ascontiguousarray(h1i[s] * SH).astype(bf)
        m["h0t8_init"] = a_dr(h0i[s], SH)
        m["h1t8_init"] = a_dr(h1i[s], SH)
        m["x0t8_init"] = a_dr(x0[s], SX)[:, 0]
        m["c0rz"] = c0rz[s].copy()
        m["c0n"] = c0n[s].copy()
        in_maps.append(m)
    return in_maps


def _install_neff_cache():
    """Cache walrus-compiled NEFFs keyed by BIR hash (compile is minutes)."""
    import hashlib
    import shutil
    import concourse.bass_utils as bu
    import concourse.bass2jax as b2j

    if getattr(bu, "_decoder_neff_cache", False):
        return
    orig = bu.compile_bir_kernel

    def cached(bir_json, tmpdir, neff_name="file.neff"):
        try:
            h = hashlib.sha256(bir_json).hexdigest()[:32]
            cdir = os.path.join(os.path.expanduser("~"), ".cache", "bass_neff")
            os.makedirs(cdir, exist_ok=True)
            cpath = os.path.join(cdir, h + ".neff")
            if os.path.exists(cpath):
                dst = os.path.join(tmpdir, "sg00")
                os.makedirs(dst, exist_ok=True)
                out = os.path.join(dst, neff_name)
                shutil.copy(cpath, out)
                return out
            out = orig(bir_json, tmpdir, neff_name)
            shutil.copy(out, cpath)
            return out
        except Exception:
            return orig(bir_json, tmpdir, neff_name)

    bu.compile_bir_kernel = cached
    b2j.compile_bir_kernel = cached
    bu._decoder_neff_cache = True


def kernel(**inputs):
    from concourse.bass_utils import run_bass_kernel_spmd

    _install_neff_cache()
    key = (T, 8)
    if key not in _prog_cache:
        _prog_cache[key] = _build_program(T, unroll=8)
    nc = _prog_cache[key]

    in_maps = _host_prep(inputs)
    out = run_bass_kernel_spmd(nc, in_maps, core_ids=list(range(NCORES)))
    res = np.concatenate(
        [r["res"].reshape(T, P, D).transpose(1, 0, 2) for r in out.results],
        axis=0)
    return np.ascontiguousarray(res, dtype=np.float32)
